# revision 3
# baseline (speedup 1.0000x reference)
"""Trainium2 Bass kernel for the AttentionLoop module.

Reference computation (S=2048, B=32, D=1024, E=1024):
    h = tanh(einsum('sbd,ed->sbe', dec + enc, W_fc))
    scores = einsum('sbe,e->bs', h, score_w[:,0])
    attn = softmax(scores, axis=1)          # over seq
    out = einsum('bs,sbd->bd', attn, enc)   # (B, D)

Strategy: data-parallel over batch across 8 NeuronCores (4 batches/core).
Everything is core-local, no collectives.

Per-core kernel (b-outer ordering, all heavy matmuls in float32r):
  - decW[e,b] = (W_fc dec^T)[e,b] computed once; folded into tanh as ACT bias
    (linearity: (dec+enc) @ W^T = enc @ W^T + dec @ W^T).
  - pass 1 per (b, s-block of 512): h^T tile (128e x 512s) accumulated over
    8 d-chunks in PSUM; ScalarE evacuates with fused bias+tanh to fp32r.
    Score matvec col-form: out(128s, 2) = h^T_chunk.T @ [sw | 0]
    (N=2 because fp32r requires an even moving free dim).
  - softmax without max-subtraction (scores are O(1); exp is safe in fp32):
    p = exp(scores) (fp32r), l = sum(p) via DVE strided reduce + ones-matmul.
  - pass 2 per (b, s-chunk of 128): out(1,1024) += p_chunk.T @ enc_chunk,
    final evacuation scaled by 1/l.
"""

import numpy as np

S, B, D, E = 2048, 32, 1024, 1024
NCORES = 8
BLOC = B // NCORES          # 4 batches per core
P = 128                     # partitions
DC = D // P                 # 8 d-chunks
EC = E // P                 # 8 e-chunks
SB = 512                    # s-block (moving free dim of main matmul)
NSBLK = S // SB             # 4 s-blocks per batch
NSC = S // P                # 16 s-chunks per batch

_compiled = None            # cached (nc,) program


def _build_program():
    import concourse.bacc as bacc
    import concourse.mybir as mybir
    import concourse.tile as tile

    f32 = mybir.dt.float32
    f32r = mybir.dt.float32r
    Tanh = mybir.ActivationFunctionType.Tanh
    Exp = mybir.ActivationFunctionType.Exp
    Copy = mybir.ActivationFunctionType.Copy

    nc = bacc.Bacc("TRN2", target_bir_lowering=False, debug=False,
                   num_devices=NCORES)

    enc_t = nc.declare_dram_parameter("enc_t", [D, BLOC, S], f32r, isOutput=False)
    enc_n = nc.declare_dram_parameter("enc_n", [S, BLOC, D], f32r, isOutput=False)
    dec_t = nc.declare_dram_parameter("dec_t", [D, BLOC], f32r, isOutput=False)
    w_t = nc.declare_dram_parameter("w_t", [D, E], f32r, isOutput=False)
    sw2 = nc.declare_dram_parameter("sw2", [P, EC * 2], f32r, isOutput=False)
    out_d = nc.declare_dram_parameter("out", [BLOC, D], f32, isOutput=True)

    with tile.TileContext(nc) as tc:
        with tc.tile_pool(name="const", bufs=1) as const, \
             tc.tile_pool(name="et", bufs=3) as et_pool, \
             tc.tile_pool(name="h", bufs=2) as h_pool, \
             tc.tile_pool(name="en", bufs=3) as en_pool, \
             tc.tile_pool(name="misc", bufs=2) as misc, \
             tc.tile_pool(name="ph", bufs=2, space="PSUM") as ph_pool, \
             tc.tile_pool(name="pscore", bufs=2, space="PSUM") as ps_pool, \
             tc.tile_pool(name="pout", bufs=1, space="PSUM") as po_pool, \
             tc.tile_pool(name="psmall", bufs=1, space="PSUM") as psmall:

            # ---- constants / weights ----
            w_sb = const.tile([P, DC, E], f32r)
            nc.sync.dma_start(w_sb[:], w_t.ap().rearrange("(dc p) e -> p dc e", p=P))
            dect_sb = const.tile([P, DC, BLOC], f32r)
            nc.sync.dma_start(dect_sb[:], dec_t.ap().rearrange("(dc p) b -> p dc b", p=P))
            sw_sb = const.tile([P, EC, 2], f32r)
            nc.sync.dma_start(sw_sb[:], sw2.ap().rearrange("p (ec two) -> p ec two", two=2))
            ones_sb = const.tile([P, 1], f32)
            nc.vector.memset(ones_sb[:], 1.0)

            # ---- decW^T[e, b] = sum_d W_T[d, e] * dec^T[d, b] ----
            decw_sb = const.tile([P, EC, BLOC], f32)
            for ec in range(EC):
                pdw = psmall.tile([P, BLOC], f32, tag="pdw")
                for dc in range(DC):
                    nc.tensor.matmul(
                        pdw[:], w_sb[:, dc, ec * P:(ec + 1) * P], dect_sb[:, dc, :],
                        start=(dc == 0), stop=(dc == DC - 1))
                nc.scalar.copy(decw_sb[:, ec, :], pdw[:])

            enc_t_r = enc_t.ap().rearrange("(dc p) b s -> p dc b s", p=P)
            enc_n_r = enc_n.ap().rearrange("(sc p) b d -> p sc b d", p=P)

            for b in range(BLOC):
                # ---------- pass 1: scores ----------
                ps = ps_pool.tile([P, 2 * NSC], f32)
                for sblk in range(NSBLK):
                    et = et_pool.tile([P, DC, SB], f32r)
                    nc.sync.dma_start(
                        et[:], enc_t_r[:, :, b, sblk * SB:(sblk + 1) * SB])
                    h = h_pool.tile([P, EC, SB], f32r)
                    for ec in range(EC):
                        phh = ph_pool.tile([P, SB], f32)
                        for dc in range(DC):
                            nc.tensor.matmul(
                                phh[:], w_sb[:, dc, ec * P:(ec + 1) * P],
                                et[:, dc, :],
                                start=(dc == 0), stop=(dc == DC - 1))
                        # h^T = tanh(h_pre + decW[:,b]) fused on ScalarE
                        nc.scalar.activation(h[:, ec, :], phh[:], Tanh,
                                             bias=decw_sb[:, ec, b:b + 1])
                    # score matvec: columns for the 4 s-chunks of this block
                    for j in range(SB // P):
                        sc = sblk * (SB // P) + j
                        for ec in range(EC):
                            nc.tensor.matmul(
                                ps[:, 2 * sc:2 * sc + 2],
                                h[:, ec, j * P:(j + 1) * P], sw_sb[:, ec, :],
                                start=(ec == 0), stop=(ec == EC - 1))

                # ---------- softmax (no max-subtraction; scores O(1)) ----------
                p_sb = misc.tile([P, 2 * NSC], f32r, tag="p")
                nc.scalar.activation(p_sb[:], ps[:], Exp)
                acc = misc.tile([P, 1], f32, tag="acc")
                nc.vector.tensor_reduce(
                    acc[:], p_sb[:].rearrange("p (sc two) -> p sc two", two=2)[:, :, 0],
                    mybir.AxisListType.X, mybir.AluOpType.add)
                pl = psmall.tile([1, 1], f32, tag="pl")
                nc.tensor.matmul(pl[:], acc[:], ones_sb[:], start=True, stop=True)
                l_sb = misc.tile([1, 1], f32, tag="l")
                nc.scalar.copy(l_sb[:], pl[:])
                inv_l = misc.tile([1, 1], f32, tag="invl")
                nc.vector.reciprocal(inv_l[:], l_sb[:])

                # ---------- pass 2: weighted sum over seq ----------
                po = po_pool.tile([1, D], f32)
                for sc in range(NSC):
                    en = en_pool.tile([P, D], f32r)
                    nc.sync.dma_start(en[:], enc_n_r[:, sc, b, :])
                    for g in range(D // SB):
                        nc.tensor.matmul(
                            po[0:1, g * SB:(g + 1) * SB],
                            p_sb[:, 2 * sc:2 * sc + 1], en[:, g * SB:(g + 1) * SB],
                            start=(sc == 0), stop=(sc == NSC - 1))
                out_sb = misc.tile([1, D], f32, tag="out")
                nc.scalar.activation(out_sb[:], po[:], Copy, scale=inv_l[:])
                nc.sync.dma_start(out_d.ap()[b:b + 1, :], out_sb[:])

    nc.compile()
    return nc


def _get_program():
    global _compiled
    if _compiled is None:
        _compiled = _build_program()
    return _compiled


def kernel(encoder_states, decoder_state, W_fc, score_w):
    from concourse.bass_utils import run_bass_kernel_spmd

    enc = np.asarray(encoder_states, dtype=np.float32)
    dec = np.asarray(decoder_state, dtype=np.float32)
    wfc = np.asarray(W_fc, dtype=np.float32)
    sw = np.asarray(score_w, dtype=np.float32)

    w_t = np.ascontiguousarray(wfc.T)                      # (D, E)
    sw2 = np.zeros((P, EC * 2), dtype=np.float32)
    sw2[:, 0::2] = sw[:, 0].reshape(EC, P).T

    in_maps = []
    for i in range(NCORES):
        b0 = i * BLOC
        sl = enc[:, b0:b0 + BLOC, :]
        in_maps.append({
            "enc_t": np.ascontiguousarray(sl.transpose(2, 1, 0)),  # (D, BLOC, S)
            "enc_n": np.ascontiguousarray(sl),                     # (S, BLOC, D)
            "dec_t": np.ascontiguousarray(dec[b0:b0 + BLOC, :].T), # (D, BLOC)
            "w_t": w_t,
            "sw2": sw2,
        })

    nc = _get_program()
    res = run_bass_kernel_spmd(nc, in_maps, list(range(NCORES)))
    return np.concatenate([res.results[i]["out"] for i in range(NCORES)], axis=0)


# revision 7
# speedup vs baseline: 1.3631x; 1.3631x over previous
"""Trainium2 Bass kernel for the AttentionLoop module.

Reference computation (S=2048, B=32, D=1024, E=1024):
    h = tanh(einsum('sbd,ed->sbe', dec + enc, W_fc))
    scores = einsum('sbe,e->bs', h, score_w[:,0])
    attn = softmax(scores, axis=1)          # over seq
    out = einsum('bs,sbd->bd', attn, enc)   # (B, D)

Strategy: data-parallel over batch across 8 NeuronCores (4 batches/core),
core-local, no collectives.

Per-core kernel (v2). All heavy matmuls in float32r (1 cyc/row, full fp32
bits, ~1.5e-4 matmul rel err). The main matmul makes h (s, e)-oriented:
stationary = encT s-chunk (K=d, M=s), moving = W_T (K=d, N=e), so that
  - the decoder bias (decW[b, e], varies along e) is added by a VectorE
    tensor_add on the PSUM tile against a pre-broadcast decw tile,
  - scores[s] = sum_e h[s,e] sw[e] is one fused VectorE multiply-reduce
    per s-chunk (accum_out), landing directly as a column (s on
    partitions) - no PE weight-load spam, no transposes,
  - exp runs per column as soon as its scores are ready, so the pass-2
    weighted-sum matmuls (p column stationary, natural-enc moving)
    interleave with pass-1 and keep the TensorE dense (HAM stays warm).
decW itself is computed on-device in broadcast form: the host replicates
dec^T columns 128x so the small matmul emits (128, e) tiles whose rows all
equal decW[b, :]. Softmax skips max-subtraction (scores are O(1); exp is
safe in fp32). l = sum(p) via DVE reduce + a ones-matmul partition sum;
1/l is folded into the final PSUM evacuation.
"""

import numpy as np

S, B, D, E = 2048, 32, 1024, 1024
NCORES = 8
BLOC = B // NCORES          # 4 batches per core
P = 128                     # partitions
DC = D // P                 # 8 d-chunks
SB = 512                    # moving free dim (PSUM bank)
NSBLK = S // SB             # 4 s-blocks per batch
NSC = S // P                # 16 s-chunks per batch

_compiled = None


def _build_program():
    import concourse.bacc as bacc
    import concourse.mybir as mybir
    import concourse.tile as tile

    f32 = mybir.dt.float32
    f32r = mybir.dt.float32r
    AF = mybir.ActivationFunctionType

    nc = bacc.Bacc("TRN2", target_bir_lowering=False, debug=False,
                   num_devices=NCORES)

    enc_t = nc.declare_dram_parameter("enc_t", [D, BLOC, S], f32r, isOutput=False)
    enc_n = nc.declare_dram_parameter("enc_n", [S, BLOC, D], f32r, isOutput=False)
    dec_rep = nc.declare_dram_parameter("dec_rep", [D, BLOC * P], f32r, isOutput=False)
    w_t = nc.declare_dram_parameter("w_t", [D, E], f32r, isOutput=False)
    sw_bc = nc.declare_dram_parameter("sw_bc", [P, E], f32, isOutput=False)
    out_d = nc.declare_dram_parameter("out", [BLOC, D], f32, isOutput=True)

    with tile.TileContext(nc) as tc:
        with tc.tile_pool(name="const", bufs=1) as const, \
             tc.tile_pool(name="et", bufs=3) as et_pool, \
             tc.tile_pool(name="h", bufs=3) as h_pool, \
             tc.tile_pool(name="en", bufs=4) as en_pool, \
             tc.tile_pool(name="misc", bufs=2) as misc, \
             tc.tile_pool(name="ph", bufs=4, space="PSUM") as ph_pool, \
             tc.tile_pool(name="pout", bufs=1, space="PSUM") as po_pool, \
             tc.tile_pool(name="psmall", bufs=2, space="PSUM") as psmall:

            # ---- constants / weights ----
            w_sb = const.tile([P, DC, E], f32r)
            nc.sync.dma_start(w_sb[:], w_t.ap().rearrange("(dc p) e -> p dc e", p=P))
            decr_sb = const.tile([P, DC, BLOC * P], f32r)
            nc.sync.dma_start(decr_sb[:],
                              dec_rep.ap().rearrange("(dc p) m -> p dc m", p=P))
            swbc_sb = const.tile([P, E], f32)
            nc.sync.dma_start(swbc_sb[:], sw_bc.ap())
            ones_sb = const.tile([P, 1], f32)
            nc.vector.memset(ones_sb[:], 1.0)

            # ---- decW in broadcast form: decw_bc[b][p, e] = decW[b, e] ----
            decw_bc = const.tile([P, BLOC, E], f32)
            for b in range(BLOC):
                for g in range(E // SB):
                    pdw = ph_pool.tile([P, SB], f32, tag="phh")
                    for dc in range(DC):
                        nc.tensor.matmul(
                            pdw[:], decr_sb[:, dc, b * P:(b + 1) * P],
                            w_sb[:, dc, g * SB:(g + 1) * SB],
                            start=(dc == 0), stop=(dc == DC - 1))
                    nc.scalar.copy(decw_bc[:, b, g * SB:(g + 1) * SB], pdw[:])

            enc_t_r = enc_t.ap().rearrange("(dc p) b s -> p dc b s", p=P)
            enc_n_r = enc_n.ap().rearrange("(sc p) b d -> p sc b d", p=P)

            for b in range(BLOC):
                scores = misc.tile([P, NSC], f32, tag="scores")
                p_sb = misc.tile([P, NSC], f32r, tag="p")
                po = po_pool.tile([1, D], f32)
                for sblk in range(NSBLK):
                    et = et_pool.tile([P, DC, SB], f32r)
                    nc.sync.dma_start(
                        et[:], enc_t_r[:, :, b, sblk * SB:(sblk + 1) * SB])
                    for j in range(SB // P):
                        sc = sblk * (SB // P) + j
                        h = h_pool.tile([P, E], f32)
                        for g in range(E // SB):
                            phh = ph_pool.tile([P, SB], f32)
                            for dc in range(DC):
                                nc.tensor.matmul(
                                    phh[:], et[:, dc, j * P:(j + 1) * P],
                                    w_sb[:, dc, g * SB:(g + 1) * SB],
                                    start=(dc == 0), stop=(dc == DC - 1))
                            # + decW[b, e] (varies along free dim -> DVE add)
                            nc.vector.tensor_add(
                                phh[:], phh[:], decw_bc[:, b, g * SB:(g + 1) * SB])
                            nc.scalar.activation(
                                h[:, g * SB:(g + 1) * SB], phh[:], AF.Tanh)
                        # scores[s] = sum_e h[s,e]*sw[e]: fused DVE mul+reduce
                        g_scr = misc.tile([P, E], f32, tag="scratch")
                        nc.vector.scalar_tensor_tensor(
                            g_scr[:], h[:], 1.0, swbc_sb[:],
                            mybir.AluOpType.mult, mybir.AluOpType.mult,
                            accum_out=scores[:, sc:sc + 1])
                        nc.scalar.activation(p_sb[:, sc:sc + 1],
                                             scores[:, sc:sc + 1], AF.Exp)
                        # pass-2: po += p_col.T @ enc_chunk (unnormalized)
                        en = en_pool.tile([P, D], f32r)
                        nc.sync.dma_start(en[:], enc_n_r[:, sc, b, :])
                        for g in range(D // SB):
                            nc.tensor.matmul(
                                po[0:1, g * SB:(g + 1) * SB],
                                p_sb[:, sc:sc + 1], en[:, g * SB:(g + 1) * SB],
                                start=(sc == 0), stop=(sc == NSC - 1))

                # ---- softmax denominator and final evacuation ----
                acc = misc.tile([P, 1], f32, tag="acc")
                nc.vector.tensor_reduce(acc[:], p_sb[:], mybir.AxisListType.X,
                                        mybir.AluOpType.add)
                pl = psmall.tile([1, 1], f32, tag="pl")
                nc.tensor.matmul(pl[:], acc[:], ones_sb[:], start=True, stop=True)
                l_sb = misc.tile([1, 1], f32, tag="l")
                nc.scalar.copy(l_sb[:], pl[:])
                inv_l = misc.tile([1, 1], f32, tag="invl")
                nc.vector.reciprocal(inv_l[:], l_sb[:])
                out_sb = misc.tile([1, D], f32, tag="out")
                nc.scalar.activation(out_sb[:], po[:], AF.Copy, scale=inv_l[:])
                nc.sync.dma_start(out_d.ap()[b:b + 1, :], out_sb[:])

    nc.compile()
    return nc


def _get_program():
    global _compiled
    if _compiled is None:
        _compiled = _build_program()
    return _compiled


def make_in_maps(encoder_states, decoder_state, W_fc, score_w):
    """Shard + lay out full inputs into per-core input maps."""
    enc = np.asarray(encoder_states, dtype=np.float32)
    dec = np.asarray(decoder_state, dtype=np.float32)
    wfc = np.asarray(W_fc, dtype=np.float32)
    sw = np.asarray(score_w, dtype=np.float32)

    w_t = np.ascontiguousarray(wfc.T)                       # (D, E)
    sw_bc = np.ascontiguousarray(
        np.broadcast_to(sw[:, 0][None, :], (P, E)))         # (P, E)

    in_maps = []
    for i in range(NCORES):
        b0 = i * BLOC
        sl = enc[:, b0:b0 + BLOC, :]
        dec_rep = np.repeat(dec[b0:b0 + BLOC, :], P, axis=0).T  # (D, BLOC*P)
        in_maps.append({
            "enc_t": np.ascontiguousarray(sl.transpose(2, 1, 0)),  # (D, BLOC, S)
            "enc_n": np.ascontiguousarray(sl),                     # (S, BLOC, D)
            "dec_rep": np.ascontiguousarray(dec_rep),
            "w_t": w_t,
            "sw_bc": sw_bc,
        })
    return in_maps


def kernel(encoder_states, decoder_state, W_fc, score_w):
    from concourse.bass_utils import run_bass_kernel_spmd

    in_maps = make_in_maps(encoder_states, decoder_state, W_fc, score_w)
    nc = _get_program()
    res = run_bass_kernel_spmd(nc, in_maps, list(range(NCORES)))
    return np.concatenate([res.results[i]["out"] for i in range(NCORES)], axis=0)


# revision 9
# speedup vs baseline: 1.4031x; 1.0294x over previous
"""Trainium2 Bass kernel for the AttentionLoop module.

Reference computation (S=2048, B=32, D=1024, E=1024):
    h = tanh(einsum('sbd,ed->sbe', dec + enc, W_fc))
    scores = einsum('sbe,e->bs', h, score_w[:,0])
    attn = softmax(scores, axis=1)          # over seq
    out = einsum('bs,sbd->bd', attn, enc)   # (B, D)

Strategy: data-parallel over batch across 8 NeuronCores (4 batches/core),
core-local, no collectives.

Per-core kernel (v3). Heavy matmuls in float32r (1 cyc/row, full fp32 bits,
~1.5e-4 matmul rel err). The main matmul makes h (s, e)-oriented:
stationary = encT s-chunk (K=d, M=s), moving = W_T (K=d, N=e):
  - dc-outer / e-half-inner matmul order so each LDWEIGHTS (fp32r weight
    load is as long as one 512-col matmul) amortizes over two matmuls,
  - decoder bias decW[b, e] (varies along e) is a VectorE tensor_add on the
    PSUM tile against a pre-broadcast decw tile; decW is computed on-device
    by one M=4 matmul chain and replicated across partitions by GpSimd
    partition_broadcast,
  - scores[s] = sum_e h[s,e] sw[e] is one fused VectorE scalar_tensor_tensor
    (mult + accum_out) per s-chunk, landing directly as a column,
  - exp runs per column as soon as its scores are ready, so the pass-2
    weighted-sum matmuls (p column stationary, natural-enc moving)
    interleave with pass-1 and keep the TensorE dense.
W and encT DMAs are split per d-chunk so the first matmul starts ~3us in.
Softmax skips max-subtraction (scores are O(1); exp is safe in fp32).
l = sum(p) via DVE reduce + ones-matmul partition sum; 1/l is folded into
the final PSUM evacuation.
"""

import numpy as np

S, B, D, E = 2048, 32, 1024, 1024
NCORES = 8
BLOC = B // NCORES          # 4 batches per core
P = 128                     # partitions
DC = D // P                 # 8 d-chunks
SB = 512                    # moving free dim (PSUM bank)
NSBLK = S // SB             # 4 s-blocks per batch
NSC = S // P                # 16 s-chunks per batch

_compiled = None


def _build_program():
    import concourse.bacc as bacc
    import concourse.mybir as mybir
    import concourse.tile as tile

    f32 = mybir.dt.float32
    f32r = mybir.dt.float32r
    AF = mybir.ActivationFunctionType

    nc = bacc.Bacc("TRN2", target_bir_lowering=False, debug=False,
                   num_devices=NCORES)

    enc_t = nc.declare_dram_parameter("enc_t", [D, BLOC, S], f32r, isOutput=False)
    enc_n = nc.declare_dram_parameter("enc_n", [S, BLOC, D], f32r, isOutput=False)
    dec_t = nc.declare_dram_parameter("dec_t", [D, BLOC], f32r, isOutput=False)
    w_t = nc.declare_dram_parameter("w_t", [D, E], f32r, isOutput=False)
    sw_row = nc.declare_dram_parameter("sw_row", [1, E], f32, isOutput=False)
    out_d = nc.declare_dram_parameter("out", [BLOC, D], f32, isOutput=True)

    with tile.TileContext(nc) as tc:
        with tc.tile_pool(name="const", bufs=1) as const, \
             tc.tile_pool(name="et", bufs=3) as et_pool, \
             tc.tile_pool(name="h", bufs=3) as h_pool, \
             tc.tile_pool(name="en", bufs=4) as en_pool, \
             tc.tile_pool(name="misc", bufs=2) as misc, \
             tc.tile_pool(name="ph", bufs=4, space="PSUM") as ph_pool, \
             tc.tile_pool(name="pout", bufs=1, space="PSUM") as po_pool, \
             tc.tile_pool(name="psmall", bufs=1, space="PSUM") as psmall:

            enc_t_r = enc_t.ap().rearrange("(dc p) b s -> p dc b s", p=P)
            enc_n_r = enc_n.ap().rearrange("(sc p) b d -> p sc b d", p=P)
            w_t_r = w_t.ap().rearrange("(dc p) e -> p dc e", p=P)

            # ---- first-needed DMAs up front, all split per d-chunk ----
            et0 = et_pool.tile([P, DC, SB], f32r)
            w_sb = const.tile([P, DC, E], f32r)
            for dc in range(DC):
                nc.sync.dma_start(et0[:, dc, :], enc_t_r[:, dc, 0, 0:SB])
                nc.sync.dma_start(w_sb[:, dc, :], w_t_r[:, dc, :])
            dect_sb = const.tile([P, DC, BLOC], f32r)
            nc.sync.dma_start(dect_sb[:],
                              dec_t.ap().rearrange("(dc p) b -> p dc b", p=P))
            swr_sb = const.tile([1, E], f32)
            nc.sync.dma_start(swr_sb[:], sw_row.ap())
            ones_sb = const.tile([P, 1], f32)
            nc.vector.memset(ones_sb[:], 1.0)

            # ---- sw broadcast across partitions ----
            swbc_sb = const.tile([P, E], f32)
            nc.gpsimd.partition_broadcast(swbc_sb[:], swr_sb[:])

            # ---- decW: one M=4 chain, rows extracted + broadcast ----
            decw4 = const.tile([BLOC, E], f32)
            for g in range(E // SB):
                pdw = psmall.tile([BLOC, SB], f32, tag="pdw")
                for dc in range(DC):
                    nc.tensor.matmul(
                        pdw[:], dect_sb[:, dc, :], w_sb[:, dc, g * SB:(g + 1) * SB],
                        start=(dc == 0), stop=(dc == DC - 1))
                nc.scalar.copy(decw4[:, g * SB:(g + 1) * SB], pdw[:])
            decw_bc = const.tile([P, BLOC, E], f32)
            for b in range(BLOC):
                row = const.tile([1, E], f32, tag=f"dwrow{b}")
                nc.sync.dma_start(row[:], decw4[b:b + 1, :])
                nc.gpsimd.partition_broadcast(decw_bc[:, b, :], row[:])

            for b in range(BLOC):
                scores = misc.tile([P, NSC], f32, tag="scores")
                p_sb = misc.tile([P, NSC], f32r, tag="p")
                po = po_pool.tile([1, D], f32)
                for sblk in range(NSBLK):
                    if b == 0 and sblk == 0:
                        et = et0
                    else:
                        et = et_pool.tile([P, DC, SB], f32r, tag="et0")
                        for dc in range(DC):
                            nc.sync.dma_start(
                                et[:, dc, :],
                                enc_t_r[:, dc, b, sblk * SB:(sblk + 1) * SB])
                    for j in range(SB // P):
                        sc = sblk * (SB // P) + j
                        h = h_pool.tile([P, E], f32)
                        phh = [ph_pool.tile([P, SB], f32, tag="phh",
                                            name=f"phh{g}")
                               for g in range(E // SB)]
                        for dc in range(DC):
                            for g in range(E // SB):
                                nc.tensor.matmul(
                                    phh[g][:], et[:, dc, j * P:(j + 1) * P],
                                    w_sb[:, dc, g * SB:(g + 1) * SB],
                                    start=(dc == 0), stop=(dc == DC - 1))
                        for g in range(E // SB):
                            # + decW[b, e] (varies along free dim -> DVE add)
                            nc.vector.tensor_add(
                                phh[g][:], phh[g][:],
                                decw_bc[:, b, g * SB:(g + 1) * SB])
                            nc.scalar.activation(
                                h[:, g * SB:(g + 1) * SB], phh[g][:], AF.Tanh)
                        # scores col: fused DVE (h * sw) with accum_out
                        g_scr = misc.tile([P, E], f32, tag="scratch")
                        nc.vector.scalar_tensor_tensor(
                            g_scr[:], h[:], 1.0, swbc_sb[:],
                            mybir.AluOpType.mult, mybir.AluOpType.mult,
                            accum_out=scores[:, sc:sc + 1])
                        nc.scalar.activation(p_sb[:, sc:sc + 1],
                                             scores[:, sc:sc + 1], AF.Exp)
                        # pass-2: po += p_col.T @ enc_chunk (unnormalized)
                        en = en_pool.tile([P, D], f32r)
                        nc.sync.dma_start(en[:], enc_n_r[:, sc, b, :])
                        for g in range(D // SB):
                            nc.tensor.matmul(
                                po[0:1, g * SB:(g + 1) * SB],
                                p_sb[:, sc:sc + 1], en[:, g * SB:(g + 1) * SB],
                                start=(sc == 0), stop=(sc == NSC - 1))

                # ---- softmax denominator and final evacuation ----
                acc = misc.tile([P, 1], f32, tag="acc")
                nc.vector.tensor_reduce(acc[:], p_sb[:], mybir.AxisListType.X,
                                        mybir.AluOpType.add)
                pl = psmall.tile([1, 1], f32, tag="pl")
                nc.tensor.matmul(pl[:], acc[:], ones_sb[:], start=True, stop=True)
                l_sb = misc.tile([1, 1], f32, tag="l")
                nc.scalar.copy(l_sb[:], pl[:])
                inv_l = misc.tile([1, 1], f32, tag="invl")
                nc.vector.reciprocal(inv_l[:], l_sb[:])
                out_sb = misc.tile([1, D], f32, tag="out")
                nc.scalar.activation(out_sb[:], po[:], AF.Copy, scale=inv_l[:])
                nc.sync.dma_start(out_d.ap()[b:b + 1, :], out_sb[:])

    nc.compile()
    return nc


def _get_program():
    global _compiled
    if _compiled is None:
        _compiled = _build_program()
    return _compiled


def make_in_maps(encoder_states, decoder_state, W_fc, score_w):
    """Shard + lay out full inputs into per-core input maps."""
    enc = np.asarray(encoder_states, dtype=np.float32)
    dec = np.asarray(decoder_state, dtype=np.float32)
    wfc = np.asarray(W_fc, dtype=np.float32)
    sw = np.asarray(score_w, dtype=np.float32)

    w_t = np.ascontiguousarray(wfc.T)                       # (D, E)
    sw_row = np.ascontiguousarray(sw[:, 0][None, :])        # (1, E)

    in_maps = []
    for i in range(NCORES):
        b0 = i * BLOC
        sl = enc[:, b0:b0 + BLOC, :]
        in_maps.append({
            "enc_t": np.ascontiguousarray(sl.transpose(2, 1, 0)),  # (D, BLOC, S)
            "enc_n": np.ascontiguousarray(sl),                     # (S, BLOC, D)
            "dec_t": np.ascontiguousarray(dec[b0:b0 + BLOC, :].T), # (D, BLOC)
            "w_t": w_t,
            "sw_row": sw_row,
        })
    return in_maps


def kernel(encoder_states, decoder_state, W_fc, score_w):
    from concourse.bass_utils import run_bass_kernel_spmd

    in_maps = make_in_maps(encoder_states, decoder_state, W_fc, score_w)
    nc = _get_program()
    res = run_bass_kernel_spmd(nc, in_maps, list(range(NCORES)))
    return np.concatenate([res.results[i]["out"] for i in range(NCORES)], axis=0)
